# revision 1
# baseline (speedup 1.0000x reference)
"""Multi-head attention (B=2, N=4096, D=512, H=8) on 8 trn2 NeuronCores.

Sharding: core c handles batch b = c//4 and head-pair p = c%4 (heads 2p,
2p+1).  Each core projects its batch's Q/K/V against its pair's weight
columns, computes transposed attention scores sT = K_h @ Q_h^T, applies
exp((1/sqrt(dk))*sT) on the ACT engine, multiplies by an augmented
V (extra ones column) so the softmax denominators fall out of the same
matmul, and applies its rows of Wo.  Normalization by the softmax
denominator commutes with the output projection (it is a per-query row
scale), so it is applied on the host during the cross-core reduction.

Device layouts (host pre-arranges):
  xt{q,k,v}: X^T            [512, N]  (D on partitions when tiled)
  w{q,k,v}:  [128, 4, 128]  w[p, dc, c] = W[dc*128+p, off+c]
  b{q,k,v}:  [128, 1]       pair slice of bias
  wo:        [128, 4, 128]  wo[p, mt, c] = Wo[off+p, mt*128+c]
Outputs per core:
  y0, y1: [512, N]  y_h[dout, q] = (O_un_h @ Wo_h)^T  (unnormalized)
  den:    [2, N]    softmax denominators per head
Final host step: out[b] = (sum_{p,h} y_h / den_h).T + bo
"""

import numpy as np

_B, _N, _D, _H, _DK = 2, 4096, 512, 8, 64
_NCORES = 8

_nc_cache = {}


def _build(n=_N, zero_bias=False):
    import concourse.mybir as mybir
    import concourse.tile as tile
    from concourse import bacc
    from concourse.masks import make_identity

    f32 = mybir.dt.float32
    f32r = mybir.dt.float32r
    bf16 = mybir.dt.float16
    Exp = mybir.ActivationFunctionType.Exp
    D = _D
    NKC = n // 128  # k chunks of 128 (columns of sT)
    NQC = n // 512  # q chunks of 512
    blocks = []
    i = 0
    while i < NKC:
        blen = min(3, NKC - i)
        blocks.append((i, blen))
        i += blen

    nc = bacc.Bacc(
        "TRN2", target_bir_lowering=False, debug=False, num_devices=_NCORES
    )

    xt = {
        t: nc.dram_tensor(f"xt{t}", [D, n], bf16, kind="ExternalInput").ap()
        for t in "qkv"
    }
    w = {
        t: nc.dram_tensor(f"w{t}", [128, 4, 128], bf16, kind="ExternalInput").ap()
        for t in "qkv"
    }
    bvec = {
        t: nc.dram_tensor(f"b{t}", [128, 1], f32, kind="ExternalInput").ap()
        for t in "qkv"
    }
    wo = nc.dram_tensor("wo", [64, 2, 4, 128], bf16, kind="ExternalInput").ap()
    y_out = [
        nc.dram_tensor(f"y{h}", [D, n], bf16, kind="ExternalOutput").ap()
        for h in range(2)
    ]
    den_out = nc.dram_tensor("den", [2, n], f32, kind="ExternalOutput").ap()

    with tile.TileContext(nc) as tc:
        with (
            tc.tile_pool(name="consts", bufs=1) as consts,
            tc.tile_pool(name="xtp", bufs=8) as xtp,
            tc.tile_pool(name="persist", bufs=1) as persist,
            tc.tile_pool(name="ep", bufs=4) as ep,
            tc.tile_pool(name="psA", bufs=2, space="PSUM") as psA,
            tc.tile_pool(name="psB", bufs=2, space="PSUM") as psB,
        ):
            ident = consts.tile([128, 128], bf16, name="ident")
            make_identity(nc, ident)
            wsb, bsb = {}, {}
            for t in "qkv":
                wsb[t] = consts.tile([128, 4, 128], bf16, name=f"w{t}sb", tag=f"w{t}sb")
                nc.sync.dma_start(out=wsb[t], in_=w[t])
                bsb[t] = consts.tile([128, 1], f32, name=f"b{t}sb", tag=f"b{t}sb")
                nc.sync.dma_start(out=bsb[t], in_=bvec[t])
            wosb = consts.tile([64, 2, 4, 128], bf16, name="wosb", tag="wosb")
            nc.sync.dma_start(out=wosb, in_=wo)

            NNC = n // 512
            qt_t = [
                persist.tile([128, 512], bf16, name=f"qt{i}", tag=f"qt{i}")
                for i in range(NNC)
            ]
            kt_t = [
                persist.tile([128, 512], bf16, name=f"kt{i}", tag=f"kt{i}")
                for i in range(NNC)
            ]
            vt_t = [
                persist.tile([128, 512], bf16, name=f"vt{i}", tag=f"vt{i}")
                for i in range(NNC)
            ]
            vch = [
                [
                    persist.tile(
                        [128, 128], bf16, name=f"vch{h}_{c}", tag=f"vch{h}_{c}"
                    )
                    for c in range(NKC)
                ]
                for h in range(2)
            ]
            ot = {
                (h, qc): persist.tile(
                    [64, 512], bf16, name=f"ot{h}_{qc}", tag=f"ot{h}_{qc}"
                )
                for h in range(2)
                for qc in range(n // 512)
            }
            den_sb = [
                persist.tile([65, n], f32, name=f"den{h}", tag=f"den{h}")
                for h in range(2)
            ]
            for h in range(2):
                for c in range(NKC):
                    nc.vector.memset(vch[h][c][:, 64:128], 0.0)
                    nc.vector.memset(vch[h][c][:, 64:65], 1.0)

            # ---- phase 1: projections  t^T = W_p^T @ X^T + b ----
            # nk-outer so early chunks of q/k/v unlock attention ASAP
            dest = {"q": qt_t, "k": kt_t, "v": vt_t}
            for nk in range(NQC):
                for t in "qkv":
                    ppsum = psA.tile([128, 512], f32, name=f"pp_{t}{nk}", tag="s")
                    for dc in range(4):
                        xtile = xtp.tile(
                            [128, 512], bf16, name=f"x_{t}{nk}{dc}", tag="xt"
                        )
                        nc.sync.dma_start(
                            out=xtile,
                            in_=xt[t][dc * 128 : (dc + 1) * 128, nk * 512 : (nk + 1) * 512],
                        )
                        nc.tensor.matmul(
                            ppsum,
                            wsb[t][:, dc, :],
                            xtile,
                            start=(dc == 0),
                            stop=(dc == 3),
                        )
                    if zero_bias:
                        nc.scalar.activation(
                            out=dest[t][nk],
                            in_=ppsum,
                            func=mybir.ActivationFunctionType.Copy,
                        )
                    else:
                        nc.vector.tensor_scalar_add(
                            out=dest[t][nk], in0=ppsum, scalar1=bsb[t]
                        )
                # v^T chunks -> per-head augmented layout
                for c in range(nk * 4, nk * 4 + 4):
                    pt = psA.tile([128, 512], bf16, name=f"pt{c}", tag="s")
                    nc.tensor.transpose(
                        pt[:, 0:128],
                        vt_t[c // 4][:, (c % 4) * 128 : (c % 4 + 1) * 128],
                        ident,
                    )
                    for h in range(2):
                        nc.vector.tensor_copy(
                            out=vch[h][c][:, 0:64], in_=pt[:, h * 64 : (h + 1) * 64]
                        )

            # ---- phase 2: attention, both heads as concurrent streams ----
            for qc in range(NQC):
                qs = slice(qc * 512, (qc + 1) * 512)
                o_ps = {
                    h: psB.tile([128, 512], f32, name=f"o_{h}_{qc}", tag="oy")
                    for h in range(2)
                }
                for k0, blen in blocks:
                    for h in range(2):
                        hp = slice(h * 64, (h + 1) * 64)
                        s_ps = psA.tile(
                            [128, blen * 512], f32, name=f"s_{h}_{qc}_{k0}", tag="s"
                        )
                        for j in range(blen):
                            kc = k0 + j
                            nc.tensor.matmul(
                                s_ps[:, j * 512 : (j + 1) * 512],
                                kt_t[kc // 4][hp, (kc % 4) * 128 : (kc % 4 + 1) * 128],
                                qt_t[qc][hp, :],
                                start=True,
                                stop=True,
                                skip_group_check=True,
                            )
                        e_sb = ep.tile(
                            [128, blen * 512], bf16, name=f"e_{h}_{qc}_{k0}", tag="e"
                        )
                        nc.scalar.activation(e_sb, s_ps, Exp, scale=0.125)
                        for j in range(blen):
                            kc = k0 + j
                            nc.tensor.matmul(
                                o_ps[h],
                                vch[h][kc],
                                e_sb[:, j * 512 : (j + 1) * 512],
                                start=(kc == 0),
                                stop=(kc == NKC - 1),
                                skip_group_check=True,
                            )
                for h in range(2):
                    nc.vector.tensor_copy(out=ot[(h, qc)], in_=o_ps[h][0:64, :])
                    nc.vector.tensor_copy(
                        out=den_sb[h][64:65, qs], in_=o_ps[h][64:65, :]
                    )

            # ---- phase 3: out-projection ----
            for qc in range(NQC):
                qs = slice(qc * 512, (qc + 1) * 512)
                for h in range(2):
                    for mt in range(4):
                        pool, tag = (psA, "s") if mt % 2 == 0 else (psB, "oy")
                        y_ps = pool.tile(
                            [128, 512], f32, name=f"y_{h}_{qc}_{mt}", tag=tag
                        )
                        nc.tensor.matmul(
                            y_ps,
                            wosb[:, h, mt, :],
                            ot[(h, qc)],
                            start=True,
                            stop=True,
                            skip_group_check=True,
                        )
                        y_sb = xtp.tile(
                            [128, 512], bf16, name=f"ysb_{h}_{qc}_{mt}", tag="ysb"
                        )
                        if mt % 2 == 0:
                            nc.vector.tensor_copy(out=y_sb, in_=y_ps)
                        else:
                            nc.scalar.copy(out=y_sb, in_=y_ps)
                        nc.sync.dma_start(
                            out=y_out[h][mt * 128 : (mt + 1) * 128, qs], in_=y_sb
                        )
            for h in range(2):
                nc.sync.dma_start(
                    out=den_out[h : h + 1, :], in_=den_sb[h][64:65, :]
                )
    nc.finalize()
    return nc


def get_nc(n=_N, zero_bias=False):
    key = (n, zero_bias)
    if key not in _nc_cache:
        _nc_cache[key] = _build(n, zero_bias)
    return _nc_cache[key]


def make_in_maps(Q, K, V, Wq, bq, Wk, bk, Wv, bv, Wo, bo, n=_N):
    """Per-core input dicts (host-side sharding / layout prep)."""
    bf = np.float16
    xts = {}
    for b in range(_B):
        xts[b] = {
            "xtq": np.ascontiguousarray(Q[b][:n].T.astype(bf)),
            "xtk": np.ascontiguousarray(K[b][:n].T.astype(bf)),
            "xtv": np.ascontiguousarray(V[b][:n].T.astype(bf)),
        }
    in_maps = []
    for c in range(_NCORES):
        b, p = divmod(c, 4)
        off = p * 128
        m = dict(xts[b])
        for t, W, bias in (("q", Wq, bq), ("k", Wk, bk), ("v", Wv, bv)):
            m[f"w{t}"] = np.ascontiguousarray(
                W[:, off : off + 128].reshape(4, 128, 128).transpose(1, 0, 2).astype(bf)
            )
            m[f"b{t}"] = np.ascontiguousarray(bias[off : off + 128].reshape(128, 1))
        m["wo"] = np.ascontiguousarray(
            Wo[off : off + 128].reshape(2, 64, 4, 128).transpose(1, 0, 2, 3).astype(bf)
        )
        in_maps.append(m)
    return in_maps


def assemble(results, bo, n=_N):
    """Cross-core reduction: normalize by softmax denominators, sum heads,
    add output bias, restore [B, N, D] layout."""
    out = np.empty((_B, n, _D), np.float32)
    for b in range(_B):
        acc = np.zeros((_D, n), np.float32)
        for p in range(4):
            r = results[4 * b + p]
            for h in range(2):
                acc += r[f"y{h}"].astype(np.float32) / r["den"][h][None, :]
        out[b] = acc.T + bo
    return out


def kernel(Q, K, V, Wq, bq, Wk, bk, Wv, bv, Wo, bo):
    from concourse import bass_utils

    args = [np.asarray(a, np.float32) for a in (Q, K, V, Wq, bq, Wk, bk, Wv, bv, Wo, bo)]
    Q, K, V, Wq, bq, Wk, bk, Wv, bv, Wo, bo = args
    zb = not (np.any(bq) or np.any(bk) or np.any(bv))
    nc = get_nc(zero_bias=zb)
    in_maps = make_in_maps(Q, K, V, Wq, bq, Wk, bk, Wv, bv, Wo, bo)
    res = bass_utils.run_bass_kernel_spmd(
        nc, in_maps, core_ids=list(range(_NCORES))
    )
    return assemble(res.results, bo)



# revision 3
# speedup vs baseline: 1.4177x; 1.4177x over previous
"""Multi-head attention (B=2, N=4096, D=512, H=8) on 8 trn2 NeuronCores.

Sharding: core c handles batch b = c//4 and head-pair p = c%4 (heads 2p,
2p+1).  Each core projects its batch's Q/K/V against its pair's weight
columns, computes transposed attention scores sT = K_h @ Q_h^T, applies
exp((1/sqrt(dk))*sT) on the ACT engine, multiplies by an augmented
V (extra ones column) so the softmax denominators fall out of the same
matmul, and applies its rows of Wo.  Normalization by the softmax
denominator commutes with the output projection (it is a per-query row
scale), so it is applied on the host during the cross-core reduction.

v2 pipeline structure (vs v1):
  - the two heads' score matmuls for each key chunk are issued
    back-to-back; their contraction dims live on disjoint partition
    ranges (0-63 / 64-127) so the PE array runs them concurrently via
    row-group tiling.
  - one exp ACTIVATE per key chunk covers both heads ([128, 1024]).
  - out-projection + output DMA are interleaved per query chunk
    (no serial tail), and q projections are interleaved with attention
    so only the k/v projections gate the pipeline start.
  - inputs are fetched with one 3D DMA per (tensor, 512-chunk).

Device layouts (host pre-arranges):
  xt{q,k,v}: X^T            [512, N]  (D on partitions when tiled)
  w{q,k,v}:  [128, 4, 128]  w[p, dc, c] = W[dc*128+p, off+c]
  b{q,k,v}:  [128, 1]       pair slice of bias
  wo:        [64, 2, 4, 128] wo[p, h, mt, c] = Wo[off+h*64+p, mt*128+c]
Outputs per core:
  y0, y1: [512, N]  y_h[dout, q] = (O_un_h @ Wo_h)^T  (unnormalized)
  den:    [2, N]    softmax denominators per head
Final host step: out[b] = (sum_{p,h} y_h / den_h).T + bo
"""

import numpy as np

_B, _N, _D, _H, _DK = 2, 4096, 512, 8, 64
_NCORES = 8

_nc_cache = {}


def _build(n=_N, zero_bias=False):
    import concourse.mybir as mybir
    import concourse.tile as tile
    from concourse import bacc
    from concourse.masks import make_identity

    f32 = mybir.dt.float32
    bf16 = mybir.dt.float16
    Exp = mybir.ActivationFunctionType.Exp
    Copy = mybir.ActivationFunctionType.Copy
    D = _D
    NKC = n // 128  # key chunks of 128 (partition dim of sT)
    NQC = n // 512  # query chunks of 512

    nc = bacc.Bacc(
        "TRN2", target_bir_lowering=False, debug=False, num_devices=_NCORES
    )

    xt = {
        t: nc.dram_tensor(f"xt{t}", [D, n], bf16, kind="ExternalInput").ap()
        for t in "qkv"
    }
    w = {
        t: nc.dram_tensor(f"w{t}", [128, 4, 128], bf16, kind="ExternalInput").ap()
        for t in "qkv"
    }
    bvec = {
        t: nc.dram_tensor(f"b{t}", [128, 1], f32, kind="ExternalInput").ap()
        for t in "qkv"
    }
    wo = nc.dram_tensor("wo", [64, 2, 4, 128], bf16, kind="ExternalInput").ap()
    y_out = [
        nc.dram_tensor(f"y{h}", [D, n], bf16, kind="ExternalOutput").ap()
        for h in range(2)
    ]
    den_out = nc.dram_tensor("den", [2, n], f32, kind="ExternalOutput").ap()

    with tile.TileContext(nc) as tc:
        with (
            tc.tile_pool(name="consts", bufs=1) as consts,
            tc.tile_pool(name="xtp", bufs=3) as xtp,
            tc.tile_pool(name="persist", bufs=1) as persist,
            tc.tile_pool(name="otp", bufs=2) as otp,
            tc.tile_pool(name="ysbp", bufs=2) as ysbp,
            tc.tile_pool(name="ep", bufs=6) as ep,
            tc.tile_pool(name="psA", bufs=2, space="PSUM") as psA,
            tc.tile_pool(name="psB", bufs=1, space="PSUM") as psB,
        ):
            ident = consts.tile([128, 128], bf16, name="ident")
            make_identity(nc, ident)
            wsb, bsb = {}, {}
            for t in "qkv":
                wsb[t] = consts.tile([128, 4, 128], bf16, name=f"w{t}sb", tag=f"w{t}sb")
                nc.sync.dma_start(out=wsb[t], in_=w[t])
                if not zero_bias:
                    bsb[t] = consts.tile([128, 1], f32, name=f"b{t}sb", tag=f"b{t}sb")
                    nc.sync.dma_start(out=bsb[t], in_=bvec[t])
            wosb = consts.tile([64, 2, 4, 128], bf16, name="wosb", tag="wosb")
            nc.sync.dma_start(out=wosb, in_=wo)

            # warm the ACT exp table during phase 1 (one-time ~2.7us load)
            warm = consts.tile([128, 1], f32, name="actwarm", tag="actwarm")
            nc.vector.memset(warm, 0.0)
            nc.scalar.activation(warm, warm, Exp)

            qt_t = [
                persist.tile([128, 512], bf16, name=f"qt{i}", tag=f"qt{i}")
                for i in range(NQC)
            ]
            kt_t = [
                persist.tile([128, 512], bf16, name=f"kt{i}", tag=f"kt{i}")
                for i in range(NQC)
            ]
            vt_t = [
                persist.tile([128, 512], bf16, name=f"vt{i}", tag=f"vt{i}")
                for i in range(NQC)
            ]
            # augmented V chunks: vch[:, kc, h, 0:64] = V_h[kc*128:+128, :]^T^T
            # (keys on partitions), vch[:, kc, h, 64] = 1.0 (denominator col)
            vch = persist.tile([128, NKC, 2, 65], bf16, name="vch", tag="vch")
            nc.vector.memset(vch[:, :, :, 64:65], 1.0)
            den_sb = [
                persist.tile([65, n], f32, name=f"den{h}", tag=f"den{h}")
                for h in range(2)
            ]

            def load_x(t, nk):
                xtile = xtp.tile([128, 4, 512], bf16, name=f"x_{t}{nk}", tag="xt")
                nc.sync.dma_start(
                    out=xtile,
                    in_=xt[t][:, nk * 512 : (nk + 1) * 512].rearrange(
                        "(dc p) q -> p dc q", dc=4
                    ),
                )
                return xtile

            def project(t, nk, dest):
                xtile = load_x(t, nk)
                ppsum = psA.tile([128, 512], f32, name=f"pp_{t}{nk}", tag="s")
                for dc in range(4):
                    nc.tensor.matmul(
                        ppsum,
                        wsb[t][:, dc, :],
                        xtile[:, dc, :],
                        start=(dc == 0),
                        stop=(dc == 3),
                    )
                if zero_bias:
                    nc.vector.tensor_copy(out=dest, in_=ppsum)
                else:
                    nc.vector.tensor_scalar_add(out=dest, in0=ppsum, scalar1=bsb[t])

            # ---- phase 1a: K and V projections + V transposes ----
            for nk in range(NQC):
                project("k", nk, kt_t[nk])
                project("v", nk, vt_t[nk])
                for c in range(nk * 4, nk * 4 + 4):
                    pt = psA.tile([128, 128], bf16, name=f"pt{c}", tag="s")
                    nc.tensor.transpose(
                        pt,
                        vt_t[c // 4][:, (c % 4) * 128 : (c % 4 + 1) * 128],
                        ident,
                    )
                    nc.vector.tensor_copy(
                        out=vch[:, c, :, 0:64],
                        in_=pt.rearrange("p (h d) -> p h d", h=2),
                    )

            # ---- phase 1b + 2 interleaved: q proj, attention, out-proj ----
            # out-projection of qc runs inside qc+1's kc loop (so the PE
            # queue never stalls ACT at qc boundaries); q-proj for qc+1
            # is prefetched mid-loop.
            def outproj(qc, ot):
                qs = slice(qc * 512, (qc + 1) * 512)
                y_sb = [
                    ysbp.tile([128, 4, 512], bf16, name=f"ysb{h}_{qc}", tag="ysb")
                    for h in range(2)
                ]
                for mt in range(4):
                    for h in range(2):
                        y_ps = psB.tile(
                            [128, 512], f32, name=f"y_{h}_{qc}_{mt}", tag="y", bufs=2
                        )
                        nc.tensor.matmul(
                            y_ps,
                            wosb[:, h, mt, :],
                            ot[h],
                            start=True,
                            stop=True,
                            skip_group_check=True,
                        )
                        nc.vector.tensor_copy(out=y_sb[h][:, mt, :], in_=y_ps)
                for h in range(2):
                    nc.sync.dma_start(
                        out=y_out[h][:, qs].rearrange("(mt p) q -> p mt q", mt=4),
                        in_=y_sb[h],
                    )

            project("q", 0, qt_t[0])
            prev_ot = None
            for qc in range(NQC):
                qs = slice(qc * 512, (qc + 1) * 512)
                o_ps = psB.tile(
                    [128, 2, 512], f32, name=f"o_{qc}", tag="o", bufs=1
                )
                e_tiles = {}

                def pv(kc):
                    e_sb = e_tiles.pop(kc)
                    for h in range(2):
                        nc.tensor.matmul(
                            o_ps[0:65, h, :],
                            vch[:, kc, h, :],
                            e_sb[:, h, :],
                            start=(kc == 0),
                            stop=(kc == NKC - 1),
                            skip_group_check=True,
                        )

                for kc in range(NKC):
                    s_ps = psA.tile(
                        [128, 2, 512], f32, name=f"s_{qc}_{kc}", tag="s"
                    )
                    for h in range(2):
                        hp = slice(h * 64, (h + 1) * 64)
                        nc.tensor.matmul(
                            s_ps[:, h, :],
                            kt_t[kc // 4][hp, (kc % 4) * 128 : (kc % 4 + 1) * 128],
                            qt_t[qc][hp, :],
                            start=True,
                            stop=True,
                            skip_group_check=True,
                        )
                    e_sb = ep.tile(
                        [128, 2, 512], bf16, name=f"e_{qc}_{kc}", tag="e"
                    )
                    nc.scalar.activation(e_sb, s_ps, Exp, scale=0.125)
                    e_tiles[kc] = e_sb
                    if kc >= 1:
                        pv(kc - 1)
                    if kc == 2 and prev_ot is not None:
                        outproj(qc - 1, prev_ot)
                    if kc == 16 and qc + 1 < NQC:
                        project("q", qc + 1, qt_t[qc + 1])
                pv(NKC - 1)

                # drain: O^T rows + denominators
                ot = [
                    otp.tile([64, 512], bf16, name=f"ot{h}_{qc}", tag="ot")
                    for h in range(2)
                ]
                for h in range(2):
                    nc.vector.tensor_copy(out=ot[h], in_=o_ps[0:64, h, :])
                    nc.vector.tensor_copy(
                        out=den_sb[h][64:65, qs], in_=o_ps[64:65, h, :]
                    )
                prev_ot = ot
            outproj(NQC - 1, prev_ot)
            for h in range(2):
                nc.sync.dma_start(
                    out=den_out[h : h + 1, :], in_=den_sb[h][64:65, :]
                )
    nc.finalize()
    return nc


def get_nc(n=_N, zero_bias=False):
    key = (n, zero_bias)
    if key not in _nc_cache:
        _nc_cache[key] = _build(n, zero_bias)
    return _nc_cache[key]


def make_in_maps(Q, K, V, Wq, bq, Wk, bk, Wv, bv, Wo, bo, n=_N):
    """Per-core input dicts (host-side sharding / layout prep)."""
    bf = np.float16
    xts = {}
    for b in range(_B):
        xts[b] = {
            "xtq": np.ascontiguousarray(Q[b][:n].T.astype(bf)),
            "xtk": np.ascontiguousarray(K[b][:n].T.astype(bf)),
            "xtv": np.ascontiguousarray(V[b][:n].T.astype(bf)),
        }
    in_maps = []
    for c in range(_NCORES):
        b, p = divmod(c, 4)
        off = p * 128
        m = dict(xts[b])
        for t, W, bias in (("q", Wq, bq), ("k", Wk, bk), ("v", Wv, bv)):
            m[f"w{t}"] = np.ascontiguousarray(
                W[:, off : off + 128].reshape(4, 128, 128).transpose(1, 0, 2).astype(bf)
            )
            m[f"b{t}"] = np.ascontiguousarray(bias[off : off + 128].reshape(128, 1))
        m["wo"] = np.ascontiguousarray(
            Wo[off : off + 128].reshape(2, 64, 4, 128).transpose(1, 0, 2, 3).astype(bf)
        )
        in_maps.append(m)
    return in_maps


def assemble(results, bo, n=_N):
    """Cross-core reduction: normalize by softmax denominators, sum heads,
    add output bias, restore [B, N, D] layout."""
    out = np.empty((_B, n, _D), np.float32)
    for b in range(_B):
        acc = np.zeros((_D, n), np.float32)
        for p in range(4):
            r = results[4 * b + p]
            for h in range(2):
                acc += r[f"y{h}"].astype(np.float32) / r["den"][h][None, :]
        out[b] = acc.T + bo
    return out


def kernel(Q, K, V, Wq, bq, Wk, bk, Wv, bv, Wo, bo):
    from concourse import bass_utils

    args = [np.asarray(a, np.float32) for a in (Q, K, V, Wq, bq, Wk, bk, Wv, bv, Wo, bo)]
    Q, K, V, Wq, bq, Wk, bk, Wv, bv, Wo, bo = args
    zb = not (np.any(bq) or np.any(bk) or np.any(bv))
    nc = get_nc(zero_bias=zb)
    in_maps = make_in_maps(Q, K, V, Wq, bq, Wk, bk, Wv, bv, Wo, bo)
    res = bass_utils.run_bass_kernel_spmd(
        nc, in_maps, core_ids=list(range(_NCORES))
    )
    return assemble(res.results, bo)


# revision 5
# speedup vs baseline: 1.5315x; 1.0802x over previous
"""Multi-head attention (B=2, N=4096, D=512, H=8) on 8 trn2 NeuronCores.

Sharding: core c handles batch b = c//4 and head-pair p = c%4 (heads 2p,
2p+1).  Each core projects its batch's Q/K/V against its pair's weight
columns, computes transposed attention scores sT = K_h @ Q_h^T, applies
exp((1/sqrt(dk))*sT) on the ACT engine, multiplies by an augmented
V (extra ones column) so the softmax denominators fall out of the same
matmul, and applies its rows of Wo.  Normalization by the softmax
denominator commutes with the output projection (it is a per-query row
scale), so it is applied on the host during the cross-core reduction.

v2 pipeline structure (vs v1):
  - the two heads' score matmuls for each key chunk are issued
    back-to-back; their contraction dims live on disjoint partition
    ranges (0-63 / 64-127) so the PE array runs them concurrently via
    row-group tiling.
  - one exp ACTIVATE per key chunk covers both heads ([128, 1024]).
  - out-projection + output DMA are interleaved per query chunk
    (no serial tail), and q projections are interleaved with attention
    so only the k/v projections gate the pipeline start.
  - inputs are fetched with one 3D DMA per (tensor, 512-chunk).

Device layouts (host pre-arranges):
  xt{q,k,v}: X^T            [512, N]  (D on partitions when tiled)
  w{q,k,v}:  [128, 4, 128]  w[p, dc, c] = W[dc*128+p, off+c]
  b{q,k,v}:  [128, 1]       pair slice of bias
  wo:        [64, 2, 4, 128] wo[p, h, mt, c] = Wo[off+h*64+p, mt*128+c]
Outputs per core:
  y0, y1: [512, N]  y_h[dout, q] = (O_un_h @ Wo_h)^T  (unnormalized)
  den:    [2, N]    softmax denominators per head
Final host step: out[b] = (sum_{p,h} y_h / den_h).T + bo
"""

import numpy as np

_B, _N, _D, _H, _DK = 2, 4096, 512, 8, 64
_NCORES = 8

_nc_cache = {}


def _build(n=_N, zero_bias=False):
    import concourse.mybir as mybir
    import concourse.tile as tile
    from concourse import bacc
    from concourse.masks import make_identity

    f32 = mybir.dt.float32
    bf16 = mybir.dt.float16
    Exp = mybir.ActivationFunctionType.Exp
    Copy = mybir.ActivationFunctionType.Copy
    D = _D
    NKC = n // 128  # key chunks of 128 (partition dim of sT)
    NQC = n // 512  # query chunks of 512

    nc = bacc.Bacc(
        "TRN2", target_bir_lowering=False, debug=False, num_devices=_NCORES
    )

    xt = {
        t: nc.dram_tensor(f"xt{t}", [D, n], bf16, kind="ExternalInput").ap()
        for t in "qkv"
    }
    w = {
        t: nc.dram_tensor(f"w{t}", [128, 4, 128], bf16, kind="ExternalInput").ap()
        for t in "qkv"
    }
    bvec = {
        t: nc.dram_tensor(f"b{t}", [128, 1], f32, kind="ExternalInput").ap()
        for t in "qkv"
    }
    wo = nc.dram_tensor("wo", [64, 2, 4, 128], bf16, kind="ExternalInput").ap()
    y_out = [
        nc.dram_tensor(f"y{h}", [D, n], bf16, kind="ExternalOutput").ap()
        for h in range(2)
    ]
    den_out = nc.dram_tensor("den", [2, n], f32, kind="ExternalOutput").ap()

    with tile.TileContext(nc) as tc:
        with (
            tc.tile_pool(name="consts", bufs=1) as consts,
            tc.tile_pool(name="xtp", bufs=6) as xtp,
            tc.tile_pool(name="persist", bufs=1) as persist,
            tc.tile_pool(name="otp", bufs=2) as otp,
            tc.tile_pool(name="ysbp", bufs=2) as ysbp,
            tc.tile_pool(name="ep", bufs=6) as ep,
            tc.tile_pool(name="psA", bufs=2, space="PSUM") as psA,
            tc.tile_pool(name="psB", bufs=1, space="PSUM") as psB,
        ):
            ident = consts.tile([128, 128], bf16, name="ident")
            make_identity(nc, ident)
            wsb, bsb = {}, {}
            for t in "qkv":
                wsb[t] = consts.tile([128, 4, 128], bf16, name=f"w{t}sb", tag=f"w{t}sb")
                nc.sync.dma_start(out=wsb[t], in_=w[t])
                if not zero_bias:
                    bsb[t] = consts.tile([128, 1], f32, name=f"b{t}sb", tag=f"b{t}sb")
                    nc.sync.dma_start(out=bsb[t], in_=bvec[t])
            wosb = consts.tile([64, 2, 4, 128], bf16, name="wosb", tag="wosb")
            nc.sync.dma_start(out=wosb, in_=wo)

            # warm the ACT exp table during phase 1 (one-time ~2.7us load)
            warm = consts.tile([128, 1], f32, name="actwarm", tag="actwarm")
            nc.vector.memset(warm, 0.0)
            nc.scalar.activation(warm, warm, Exp)

            qt_t = [
                persist.tile([128, 512], bf16, name=f"qt{i}", tag=f"qt{i}")
                for i in range(NQC)
            ]
            kt_t = [
                persist.tile([128, 512], bf16, name=f"kt{i}", tag=f"kt{i}")
                for i in range(NQC)
            ]
            vt_t = [
                persist.tile([128, 512], bf16, name=f"vt{i}", tag=f"vt{i}")
                for i in range(NQC)
            ]
            # augmented V chunks: vch[:, kc, h, 0:64] = V_h[kc*128:+128, :]^T^T
            # (keys on partitions), vch[:, kc, h, 64] = 1.0 (denominator col)
            vch = persist.tile([128, NKC, 2, 65], bf16, name="vch", tag="vch")
            nc.vector.memset(vch[:, :, :, 64:65], 1.0)
            den_sb = [
                persist.tile([65, n], f32, name=f"den{h}", tag=f"den{h}")
                for h in range(2)
            ]

            xtiles = {}

            def load_x(t, nk):
                xtile = xtp.tile([128, 4, 512], bf16, name=f"x_{t}{nk}", tag="xt")
                nc.sync.dma_start(
                    out=xtile,
                    in_=xt[t][:, nk * 512 : (nk + 1) * 512].rearrange(
                        "(dc p) q -> p dc q", dc=4
                    ),
                )
                xtiles[(t, nk)] = xtile

            def project(t, nk, dest):
                xtile = xtiles.pop((t, nk))
                ppsum = psA.tile([128, 512], f32, name=f"pp_{t}{nk}", tag="s")
                for dc in range(4):
                    nc.tensor.matmul(
                        ppsum,
                        wsb[t][:, dc, :],
                        xtile[:, dc, :],
                        start=(dc == 0),
                        stop=(dc == 3),
                    )
                if zero_bias:
                    nc.vector.tensor_copy(out=dest, in_=ppsum)
                else:
                    nc.vector.tensor_scalar_add(out=dest, in0=ppsum, scalar1=bsb[t])

            def transpose_v(c):
                pt = psA.tile([128, 128], bf16, name=f"pt{c}", tag="s")
                nc.tensor.transpose(
                    pt,
                    vt_t[c // 4][:, (c % 4) * 128 : (c % 4 + 1) * 128],
                    ident,
                )
                nc.vector.tensor_copy(
                    out=vch[:, c, :, 0:64],
                    in_=pt.rearrange("p (h d) -> p h d", h=2),
                )

            def outproj_step(qc, ot, mt, h):
                if (mt, h) == (0, 0):
                    self_ysb = [
                        ysbp.tile([128, 4, 512], bf16, name=f"ysb{h2}_{qc}", tag="ysb")
                        for h2 in range(2)
                    ]
                    ysb_live[qc] = self_ysb
                y_sb = ysb_live[qc]
                y_ps = psB.tile(
                    [128, 512], f32, name=f"y_{h}_{qc}_{mt}", tag="y", bufs=2
                )
                nc.tensor.matmul(
                    y_ps,
                    wosb[:, h, mt, :],
                    ot[h],
                    start=True,
                    stop=True,
                    skip_group_check=True,
                )
                nc.vector.tensor_copy(out=y_sb[h][:, mt, :], in_=y_ps)
                if (mt, h) == (3, 1):
                    qs = slice(qc * 512, (qc + 1) * 512)
                    for h2 in range(2):
                        nc.sync.dma_start(
                            out=y_out[h2][:, qs].rearrange(
                                "(mt p) q -> p mt q", mt=4
                            ),
                            in_=y_sb[h2],
                        )
                    del ysb_live[qc]

            ysb_live = {}

            # ---- fused pipeline over (qc, kc) blocks ----
            # qc 0 carries the k/v projection + transpose work injected one
            # chunk-group ahead of the scores that consume it; later qcs
            # carry the previous qc's out-projection (spread one matmul per
            # block) and the next qc's q projection.  Scores/exp run one
            # block ahead of PV so the ACT engine never waits on the PE
            # queue at block or qc boundaries.
            for t, nk in (("q", 0), ("k", 0), ("v", 0), ("k", 1), ("v", 1)):
                load_x(t, nk)
            project("q", 0, qt_t[0])
            project("k", 0, kt_t[0])
            project("v", 0, vt_t[0])
            for c in range(4):
                transpose_v(c)

            o_ps_live = {}
            e_tiles = {}
            ot_live = {}

            def emit_scores(qc, kc):
                s_ps = psA.tile([128, 2, 512], f32, name=f"s_{qc}_{kc}", tag="s")
                for h in range(2):
                    hp = slice(h * 64, (h + 1) * 64)
                    nc.tensor.matmul(
                        s_ps[:, h, :],
                        kt_t[kc // 4][hp, (kc % 4) * 128 : (kc % 4 + 1) * 128],
                        qt_t[qc][hp, :],
                        start=True,
                        stop=True,
                        skip_group_check=True,
                    )
                e_sb = ep.tile([128, 2, 512], bf16, name=f"e_{qc}_{kc}", tag="e")
                nc.scalar.activation(e_sb, s_ps, Exp, scale=0.125)
                e_tiles[(qc, kc)] = e_sb

            def emit_pv(qc, kc):
                if kc == 0:
                    o_ps_live[qc] = psB.tile(
                        [128, 2, 512], f32, name=f"o_{qc}", tag="o", bufs=1
                    )
                o_ps = o_ps_live[qc]
                e_sb = e_tiles.pop((qc, kc))
                for h in range(2):
                    nc.tensor.matmul(
                        o_ps[0:65, h, :],
                        vch[:, kc, h, :],
                        e_sb[:, h, :],
                        start=(kc == 0),
                        stop=(kc == NKC - 1),
                        skip_group_check=True,
                    )
                if kc == NKC - 1:
                    # drain: O^T rows + denominators
                    qs = slice(qc * 512, (qc + 1) * 512)
                    ot = [
                        otp.tile([64, 512], bf16, name=f"ot{h}_{qc}", tag="ot")
                        for h in range(2)
                    ]
                    for h in range(2):
                        nc.vector.tensor_copy(out=ot[h], in_=o_ps[0:64, h, :])
                        nc.vector.tensor_copy(
                            out=den_sb[h][64:65, qs], in_=o_ps[64:65, h, :]
                        )
                    ot_live[qc] = ot
                    del o_ps_live[qc]

            blocks = [(qc, kc) for qc in range(NQC) for kc in range(NKC)]
            emit_scores(0, 0)
            for i, (qc, kc) in enumerate(blocks):
                # injected side-work for this block
                if qc == 0:
                    nk = kc // 4 + 1
                    if nk < NQC:
                        step = kc % 4
                        if step == 0:
                            project("k", nk, kt_t[nk])
                        elif step == 1:
                            project("v", nk, vt_t[nk])
                        else:
                            transpose_v(4 * nk + 2 * (step - 2))
                            transpose_v(4 * nk + 2 * (step - 2) + 1)
                    nk2 = kc // 2 + 2
                    if kc % 2 == 0 and nk2 < NQC:
                        load_x("k", nk2)
                    elif kc % 2 == 1 and nk2 < NQC:
                        load_x("v", nk2)
                else:
                    if 2 <= kc < 10 and (qc - 1) in ot_live:
                        mt, h = divmod(kc - 2, 2)
                        outproj_step(qc - 1, ot_live[qc - 1], mt, h)
                        if (mt, h) == (3, 1):
                            del ot_live[qc - 1]
                if kc == 8 and qc + 1 < NQC:
                    load_x("q", qc + 1)
                if kc == 16 and qc + 1 < NQC:
                    project("q", qc + 1, qt_t[qc + 1])
                # one-block-ahead scores/exp, then this block's PV
                if i + 1 < len(blocks):
                    emit_scores(*blocks[i + 1])
                emit_pv(qc, kc)
            for mt in range(4):
                for h in range(2):
                    outproj_step(NQC - 1, ot_live[NQC - 1], mt, h)
            for h in range(2):
                nc.sync.dma_start(
                    out=den_out[h : h + 1, :], in_=den_sb[h][64:65, :]
                )
    nc.finalize()
    return nc


def get_nc(n=_N, zero_bias=False):
    key = (n, zero_bias)
    if key not in _nc_cache:
        _nc_cache[key] = _build(n, zero_bias)
    return _nc_cache[key]


def make_in_maps(Q, K, V, Wq, bq, Wk, bk, Wv, bv, Wo, bo, n=_N):
    """Per-core input dicts (host-side sharding / layout prep)."""
    bf = np.float16
    xts = {}
    for b in range(_B):
        xts[b] = {
            "xtq": np.ascontiguousarray(Q[b][:n].T.astype(bf)),
            "xtk": np.ascontiguousarray(K[b][:n].T.astype(bf)),
            "xtv": np.ascontiguousarray(V[b][:n].T.astype(bf)),
        }
    in_maps = []
    for c in range(_NCORES):
        b, p = divmod(c, 4)
        off = p * 128
        m = dict(xts[b])
        for t, W, bias in (("q", Wq, bq), ("k", Wk, bk), ("v", Wv, bv)):
            m[f"w{t}"] = np.ascontiguousarray(
                W[:, off : off + 128].reshape(4, 128, 128).transpose(1, 0, 2).astype(bf)
            )
            m[f"b{t}"] = np.ascontiguousarray(bias[off : off + 128].reshape(128, 1))
        m["wo"] = np.ascontiguousarray(
            Wo[off : off + 128].reshape(2, 64, 4, 128).transpose(1, 0, 2, 3).astype(bf)
        )
        in_maps.append(m)
    return in_maps


def assemble(results, bo, n=_N):
    """Cross-core reduction: normalize by softmax denominators, sum heads,
    add output bias, restore [B, N, D] layout."""
    out = np.empty((_B, n, _D), np.float32)
    for b in range(_B):
        acc = np.zeros((_D, n), np.float32)
        for p in range(4):
            r = results[4 * b + p]
            for h in range(2):
                acc += r[f"y{h}"].astype(np.float32) / r["den"][h][None, :]
        out[b] = acc.T + bo
    return out


def kernel(Q, K, V, Wq, bq, Wk, bk, Wv, bv, Wo, bo):
    from concourse import bass_utils

    args = [np.asarray(a, np.float32) for a in (Q, K, V, Wq, bq, Wk, bk, Wv, bv, Wo, bo)]
    Q, K, V, Wq, bq, Wk, bk, Wv, bv, Wo, bo = args
    zb = not (np.any(bq) or np.any(bk) or np.any(bv))
    nc = get_nc(zero_bias=zb)
    in_maps = make_in_maps(Q, K, V, Wq, bq, Wk, bk, Wv, bv, Wo, bo)
    res = bass_utils.run_bass_kernel_spmd(
        nc, in_maps, core_ids=list(range(_NCORES))
    )
    return assemble(res.results, bo)
